# revision 1
# baseline (speedup 1.0000x reference)
"""JointEdgeSegLoss Trainium2 kernel.

Strategy (data-parallel over batch+rows, 8 cores):
  - core k handles image n=k//2, row-half h=k%2 (384 rows = 294912 pixels).
  - On-chip per core: log-softmax denominator via ACT exp + DVE strided
    reduce; per-(image,class) partial sums of lp = x_tgt - lse via fused
    scalar_tensor_tensor (is_equal * mult with free-dim accumulate);
    histogram counts via tensor_scalar accumulate; BCE partials via
    ACT (|x|, exp, ln1p, relu) + DVE dots.
  - Host combines tiny per-core partials in float64 (the "all-reduce").

Self-contained: hardcodes all shapes; only imports the runtime (concourse).
"""

import numpy as np

import concourse.bass as bass
import concourse.bacc as bacc
import concourse.mybir as mybir
import concourse.tile as tile
from concourse import bass_utils

F32 = mybir.dt.float32
I32 = mybir.dt.int32
BF16 = mybir.dt.bfloat16
ALU = mybir.AluOpType
ACTF = mybir.ActivationFunctionType

C = 19
N, H, W = 4, 768, 768
HW = H * W                      # pixels per image
NCORES = 8
M = N * HW // NCORES            # 294912 pixels per core (half an image)
P = 128
Q = M // P                      # 2304 free elements per partition
F = 384                         # pixels-per-partition per chunk
NCH = Q // F                    # 6 chunks
EDGE_THRESH = 0.8
IGNORE = 255.0

# accumulator slot layout (per chunk): 4 class-families of 19 + 3 bce slots
SL_S1 = 0          # sum (tgt==c) * lp            [19]
SL_S2 = SL_S1 + C  # sum (tgtv==c) * lp           [19]
SL_B1 = SL_S2 + C  # sum (tgt==c)                 [19]
SL_B2 = SL_B1 + C  # sum (tgtv==c)                [19]
SL_BCE = SL_B2 + C  # [sum t*bce, sum bce, sum t] [3]
SLOTS = SL_BCE + 3  # 79
NACC = NCH * SLOTS


def build_program():
    nc = bacc.Bacc("TRN2", target_bir_lowering=False, debug=False)

    xs = nc.dram_tensor("xs", [C, P, Q], F32, kind="ExternalInput")
    ts = nc.dram_tensor("ts", [P, Q], I32, kind="ExternalInput")
    es = nc.dram_tensor("es", [P, Q], F32, kind="ExternalInput")
    ms = nc.dram_tensor("ms", [P, Q], I32, kind="ExternalInput")
    acc_d = nc.dram_tensor("acc", [P, NACC], F32, kind="ExternalOutput")

    with tile.TileContext(nc) as tc:
        with (
            tc.tile_pool(name="xp", bufs=2) as xp,
            tc.tile_pool(name="ep", bufs=2) as ep,
            tc.tile_pool(name="lpp", bufs=2) as lpp,
            tc.tile_pool(name="mp", bufs=2) as mp,
            tc.tile_pool(name="sp", bufs=2) as sp,
            tc.tile_pool(name="cst", bufs=1) as cst,
        ):
            accT = cst.tile([P, NACC], F32, tag="acc")
            junk = cst.tile([P, F], F32, tag="junk")
            junk2 = cst.tile([P, F], F32, tag="junk2")

            for k in range(NCH):
                f0 = k * F

                X = xp.tile([P, C, F], F32, tag="X")
                nc.sync.dma_start(
                    X[:], xs.ap()[:, :, f0:f0 + F].transpose([1, 0, 2])
                )
                T = mp.tile([P, F], I32, tag="T")
                nc.sync.dma_start(T[:], ts.ap()[:, f0:f0 + F])
                E = mp.tile([P, F], F32, tag="E")
                nc.sync.dma_start(E[:], es.ap()[:, f0:f0 + F])
                Mm = mp.tile([P, F], I32, tag="Mm")
                nc.sync.dma_start(Mm[:], ms.ap()[:, f0:f0 + F])

                # ---- log-softmax denominator ----
                eb = ep.tile([P, C, F], BF16, tag="eb")
                nc.scalar.activation(eb[:], X[:], ACTF.Exp)
                s = sp.tile([P, F], F32, tag="s")
                nc.vector.tensor_reduce(
                    s[:], eb[:].transpose([0, 2, 1]), axis=mybir.AxisListType.X,
                    op=ALU.add,
                )
                lse = sp.tile([P, F], F32, tag="lse")
                nc.scalar.activation(lse[:], s[:], ACTF.Ln)

                # lp = x - lse (broadcast over channel)
                lp = lpp.tile([P, C, F], F32, tag="lp")
                nc.vector.scalar_tensor_tensor(
                    lp[:], X[:], 0.0,
                    lse[:].unsqueeze(1).broadcast_to([P, C, F]),
                    op0=ALU.add, op1=ALU.subtract,
                )

                # ---- masks ----
                Tf = sp.tile([P, F], F32, tag="Tf")
                nc.vector.tensor_copy(Tf[:], T[:])
                gt = sp.tile([P, F], F32, tag="gt")
                nc.vector.tensor_scalar(
                    gt[:], E[:], EDGE_THRESH, None, op0=ALU.is_gt
                )
                # Tv = gt ? Tf : 255  ==  (Tf - 255)*gt + 255
                Tvd = sp.tile([P, F], F32, tag="Tvd")
                nc.vector.scalar_tensor_tensor(
                    Tvd[:], Tf[:], -IGNORE, gt[:],
                    op0=ALU.add, op1=ALU.mult,
                )
                Tv = sp.tile([P, F], F32, tag="Tv")
                nc.vector.tensor_scalar(
                    Tv[:], Tvd[:], IGNORE, None, op0=ALU.add
                )

                base = k * SLOTS

                def slot(j):
                    return accT[:, base + j:base + j + 1]

                # ---- per-class families ----
                for c in range(C):
                    nc.vector.scalar_tensor_tensor(
                        junk[:], Tf[:], float(c), lp[:, c, :],
                        op0=ALU.is_equal, op1=ALU.mult,
                        accum_out=slot(SL_S1 + c),
                    )
                for c in range(C):
                    nc.vector.scalar_tensor_tensor(
                        junk[:], Tv[:], float(c), lp[:, c, :],
                        op0=ALU.is_equal, op1=ALU.mult,
                        accum_out=slot(SL_S2 + c),
                    )
                for c in range(C):
                    nc.vector.tensor_scalar(
                        junk2[:], Tf[:], float(c), None, op0=ALU.is_equal,
                        op1=ALU.add, accum_out=slot(SL_B1 + c),
                    )
                for c in range(C):
                    nc.vector.tensor_scalar(
                        junk2[:], Tv[:], float(c), None, op0=ALU.is_equal,
                        op1=ALU.add, accum_out=slot(SL_B2 + c),
                    )

                # ---- bce partials ----
                tm = sp.tile([P, F], F32, tag="tm")
                nc.vector.tensor_copy(tm[:], Mm[:])
                ab = sp.tile([P, F], F32, tag="ab")
                nc.scalar.activation(ab[:], E[:], ACTF.Abs)
                en = sp.tile([P, F], F32, tag="en")
                nc.scalar.activation(en[:], ab[:], ACTF.Exp, scale=-1.0)
                l1p = sp.tile([P, F], F32, tag="l1p")
                nc.scalar.activation(l1p[:], en[:], ACTF.Ln, bias=1.0)
                r = sp.tile([P, F], F32, tag="r")
                nc.scalar.activation(r[:], E[:], ACTF.Relu)
                # bce = r + l1p - E*t
                q = sp.tile([P, F], F32, tag="q")
                nc.vector.scalar_tensor_tensor(
                    q[:], E[:], 0.0, tm[:], op0=ALU.add, op1=ALU.mult
                )
                b1 = sp.tile([P, F], F32, tag="b1")
                nc.vector.scalar_tensor_tensor(
                    b1[:], r[:], 0.0, l1p[:], op0=ALU.add, op1=ALU.add
                )
                bce = sp.tile([P, F], F32, tag="bce")
                nc.vector.scalar_tensor_tensor(
                    bce[:], b1[:], 0.0, q[:], op0=ALU.add, op1=ALU.subtract,
                    accum_out=slot(SL_BCE + 1),
                )
                nc.vector.scalar_tensor_tensor(
                    junk[:], bce[:], 0.0, tm[:], op0=ALU.add, op1=ALU.mult,
                    accum_out=slot(SL_BCE + 0),
                )
                nc.vector.tensor_scalar(
                    junk2[:], tm[:], 0.0, None, op0=ALU.add,
                    op1=ALU.add, accum_out=slot(SL_BCE + 2),
                )

            nc.sync.dma_start(acc_d.ap()[:, :], accT[:])

    nc.finalize()
    return nc


_CACHE = {}


def _get_program():
    if "nc" not in _CACHE:
        _CACHE["nc"] = build_program()
    return _CACHE["nc"]


def make_in_maps(segin, edgein, segmask, edgemask):
    in_maps = []
    for k in range(NCORES):
        n, h = k // 2, k % 2
        rs = slice(h * (H // 2), (h + 1) * (H // 2))
        in_maps.append({
            "xs": np.ascontiguousarray(
                segin[n, :, rs, :].reshape(C, P, Q)),
            "ts": np.ascontiguousarray(
                segmask[n, rs, :].reshape(P, Q)),
            "es": np.ascontiguousarray(
                edgein[n, 0, rs, :].reshape(P, Q)),
            "ms": np.ascontiguousarray(
                edgemask[n, 0, rs, :].reshape(P, Q)),
        })
    return in_maps


def combine(acc_list):
    """acc_list: per-core [P, NACC] arrays -> final f32 scalar loss."""
    # per-core partial sums over partitions+chunks, in f64
    part = np.zeros((NCORES, SLOTS))
    for k in range(NCORES):
        a = acc_list[k].astype(np.float64).reshape(P, NCH, SLOTS)
        part[k] = a.sum(axis=(0, 1))

    seg_loss = 0.0
    att_loss = 0.0
    for n in range(N):
        p = part[2 * n] + part[2 * n + 1]
        S1 = p[SL_S1:SL_S1 + C]
        S2 = p[SL_S2:SL_S2 + C]
        bins = p[SL_B1:SL_B1 + C]
        bins2 = p[SL_B2:SL_B2 + C]

        w1 = (bins != 0) * (1.0 - bins / HW) + 1.0
        seg_loss += -(w1 * S1).sum() / (w1 * bins).sum()

        vsum = bins2.sum()
        w2 = (bins2 != 0) * (1.0 - bins2 / vsum) + 1.0
        att_loss += -(w2 * S2).sum() / (w2 * bins2).sum()

    tot = part.sum(axis=0)
    pos_bce, all_bce, pos_num = (
        tot[SL_BCE + 0], tot[SL_BCE + 1], tot[SL_BCE + 2])
    cnt = float(N * HW)
    neg_num = cnt - pos_num
    neg_bce = all_bce - pos_bce
    ssum = pos_num + neg_num
    edge_loss = (neg_num / ssum * pos_bce + pos_num / ssum * neg_bce) / cnt

    return np.float32(seg_loss + 0.3 * edge_loss + 0.1 * att_loss)


def run_cores(in_maps, trace=False, **kw):
    nc = _get_program()
    res = bass_utils.run_bass_kernel_spmd(
        nc, in_maps, core_ids=list(range(NCORES)), trace=trace, **kw
    )
    return res


def kernel(segin, edgein, segmask, edgemask):
    in_maps = make_in_maps(
        np.asarray(segin), np.asarray(edgein),
        np.asarray(segmask), np.asarray(edgemask))
    res = run_cores(in_maps)
    acc_list = [out["acc"] for out in res.results]
    return combine(acc_list)



# revision 8
# speedup vs baseline: 1.5652x; 1.5652x over previous
"""JointEdgeSegLoss Trainium2 kernel.

Strategy (data-parallel over batch+rows, 8 cores):
  core k handles image n=k//2, row-half h=k%2 (294912 pixels as [128, 2304]).

Per core, streaming 6 column-chunks of 384 grouped into 3 chunk-pairs:
  - ACT: eb = exp(X) (bf16); per-pair lse = Ln(s) with Exp/Ln grouped to
    minimize activation-table reloads.
  - DVE: s = sum_C eb via bulk slice-tree adds (bf16 2x); lp = X - lse
    (one tensor_tensor per chunk, bf16 out) into pair-sized buffers.
  - Packed class key tpack = t + 32*gt (fp16): A-bins key c (edge<=thresh),
    B-bins key c+32 (edge>thresh). Count scans run as tensor_scalar
    is_equal with accum (4x perf mode); value scans sum lp per class via
    scalar_tensor_tensor is_equal*mult with accum, split across DVE/Pool.
  - BCE partials via ACT (Abs/Exp/Relu/Ln(1+x) with accum_out) + DVE dots.
  - Host combines tiny per-core [P, SLOTS] partials in float64.

Self-contained: hardcodes all shapes; only imports the runtime (concourse).
"""

import numpy as np

import concourse.bass as bass
import concourse.bacc as bacc
import concourse.mybir as mybir
import concourse.tile as tile
from concourse import bass_utils

F32 = mybir.dt.float32
I32 = mybir.dt.int32
BF16 = mybir.dt.bfloat16
FP16 = mybir.dt.float16
ALU = mybir.AluOpType
ACTF = mybir.ActivationFunctionType

C = 19
N, H, W = 4, 768, 768
HW = H * W
NCORES = 8
M = N * HW // NCORES            # 294912 pixels per core
P = 128
Q = M // P                      # 2304 columns per partition
F = 384                         # columns per chunk
NCH = Q // F                    # 6 chunks
NPAIR = NCH // 2                # 3 pair-groups (value-scan granularity)
FP = 2 * F                      # 768 columns per pair-group
EDGE_THRESH = 0.8
BKEY = 32.0                     # tpack = t + 32*gt

# how many of the 38 value scans per pair-group run on DVE (rest on Pool)
DVE_SCANS = 4

# accumulator layout (f32 slots per partition):
#   per pair-group g in 0..2:  SA[g*38 + c] (c in 0..18), SB[g*38 + 19 + c]
#   counts per half h in 0..1: NA[114 + h*38 + c], NB[114 + h*38 + 19 + c]
#   bce: +0: sum E*m, +1: sum (relu+l1p)*m, +2: sum m,
#        +3: sum relu(E), +4: sum ln1p(exp(-|E|))
SL_S = 0
SL_N = 3 * 38
SL_BCE = SL_N + 2 * 38
SLOTS = SL_BCE + 5              # 195


def build_program():
    nc = bacc.Bacc("TRN2", target_bir_lowering=False, debug=False)

    xs = nc.dram_tensor("xs", [C, P, Q], F32, kind="ExternalInput")
    ts = nc.dram_tensor("ts", [P, Q], I32, kind="ExternalInput")
    es = nc.dram_tensor("es", [P, Q], F32, kind="ExternalInput")
    ms = nc.dram_tensor("ms", [P, Q], I32, kind="ExternalInput")
    acc_d = nc.dram_tensor("acc", [P, SLOTS], F32, kind="ExternalOutput")

    with tile.TileContext(nc) as tc:
        with (
            tc.tile_pool(name="xp", bufs=2) as xp,
            tc.tile_pool(name="sp", bufs=2) as sp,
            tc.tile_pool(name="lpp", bufs=2) as lpp,
            tc.tile_pool(name="cst", bufs=1) as cst,
        ):
            accT = cst.tile([P, SLOTS], F32, tag="acc")
            Ef = cst.tile([P, Q], F32, tag="Ef")        # full edge logits
            Mf = cst.tile([P, Q], I32, tag="Mf")        # full edgemask
            tpk = cst.tile([P, Q], FP16, tag="tpk")     # t + 32*gt
            eb = cst.tile([P, C, F], BF16, tag="eb")
            t9 = cst.tile([P, 9, F], BF16, tag="t9")
            t4 = cst.tile([P, 4, F], BF16, tag="t4")
            t2 = cst.tile([P, 2, F], BF16, tag="t2")
            t1 = cst.tile([P, F], BF16, tag="t1")
            t1b = cst.tile([P, F], BF16, tag="t1b")
            junkD = cst.tile([P, FP], BF16, tag="junkD")
            junkP = cst.tile([P, FP], BF16, tag="junkP")
            junkH = cst.tile([P, Q // 2], FP16, tag="junkH")
            bA = cst.tile([P, Q], BF16, tag="bA")       # abs -> l1p
            bB = cst.tile([P, Q], BF16, tag="bB")       # exp -> bce
            bC = cst.tile([P, Q], BF16, tag="bC")       # relu

            def slot(j):
                return accT[:, j:j + 1]

            for g in range(NPAIR):
                lpT = lpp.tile([P, C, FP], BF16, tag="lp")
                Xs = [None, None]
                Ss = [None, None]

                # ---- stream the two chunks of this pair-group ----
                for u in range(2):
                    k = 2 * g + u
                    f0 = k * F
                    sl = slice(f0, f0 + F)

                    X = xp.tile([P, C, F], F32, tag="X")
                    nc.sync.dma_start(
                        X[:], xs.ap()[:, :, sl].transpose([1, 0, 2])
                    )
                    T = sp.tile([P, F], I32, tag="T")
                    nc.sync.dma_start(T[:], ts.ap()[:, sl])
                    nc.sync.dma_start(Ef[:, sl], es.ap()[:, sl])
                    nc.sync.dma_start(Mf[:, sl], ms.ap()[:, sl])
                    Xs[u] = X

                    # exp (activation table: exp set)
                    nc.scalar.activation(eb[:], X[:], ACTF.Exp)

                    # s = sum over C, bulk slice-tree in bf16 (L1 on Pool)
                    s = sp.tile([P, F], F32, tag="s")
                    nc.gpsimd.tensor_tensor(
                        t9[:], eb[:, 0:9, :], eb[:, 9:18, :], op=ALU.add)
                    nc.vector.tensor_tensor(
                        t4[:], t9[:, 0:4, :], t9[:, 4:8, :], op=ALU.add)
                    nc.vector.tensor_tensor(
                        t2[:], t4[:, 0:2, :], t4[:, 2:4, :], op=ALU.add)
                    nc.vector.tensor_tensor(
                        t1[:], t2[:, 0, :], t2[:, 1, :], op=ALU.add)
                    nc.vector.tensor_tensor(
                        t1b[:], t1[:], t9[:, 8, :], op=ALU.add)
                    nc.vector.tensor_tensor(
                        s[:], t1b[:], eb[:, 18, :], op=ALU.add)
                    Ss[u] = s

                    # masks for this chunk: gt, tpack = t + 32*gt
                    gt = sp.tile([P, F], BF16, tag="gt")
                    nc.vector.tensor_scalar(
                        gt[:], Ef[:, sl], EDGE_THRESH, None, op0=ALU.is_gt)
                    nc.vector.scalar_tensor_tensor(
                        tpk[:, sl], gt[:], BKEY, T[:],
                        op0=ALU.mult, op1=ALU.add)

                # ---- pair boundary: Ln for both chunks (one table set) ----
                lses = [None, None]
                for u in range(2):
                    lse = sp.tile([P, F], F32, tag=f"lse{u}")
                    nc.scalar.activation(lse[:], Ss[u][:], ACTF.Ln)
                    lses[u] = lse

                # lp = X - lse for both chunks, into the pair buffer (Pool)
                for u in range(2):
                    nc.gpsimd.tensor_tensor(
                        lpT[:, :, u * F:(u + 1) * F], Xs[u][:],
                        lses[u][:].unsqueeze(1).broadcast_to([P, C, F]),
                        op=ALU.subtract)

                # ---- value scans for this pair-group ----
                gsl = slice(g * FP, (g + 1) * FP)
                base = SL_S + g * 38
                scans = [(float(c), base + c) for c in range(C)]
                scans += [(float(c) + BKEY, base + 19 + c) for c in range(C)]
                for i, (key, sj) in enumerate(scans):
                    c = int(key) % 32
                    nc.vector.scalar_tensor_tensor(
                        junkD[:], tpk[:, gsl], key, lpT[:, c, :],
                        op0=ALU.is_equal, op1=ALU.mult,
                        accum_out=slot(sj))

                # ---- count scans per half (tpack-only, DVE 4x mode) ----
                # half 0 (chunks 0-2) done at g=1; half 1 (3-5) at g=2
                if g in (1, 2):
                    h = g - 1
                    hsl = slice(h * (Q // 2), (h + 1) * (Q // 2))
                    nb = SL_N + h * 38
                    for c in range(2 * C):
                        key = float(c % C) + (BKEY if c >= C else 0.0)
                        nc.vector.tensor_scalar(
                            junkH[:], tpk[:, hsl], key, None,
                            op0=ALU.is_equal, op1=ALU.add,
                            accum_out=slot(nb + c))

            # ---- bce on the full edge row ----
            nc.scalar.activation(bA[:], Ef[:], ACTF.Abs)
            nc.scalar.activation(bB[:], bA[:], ACTF.Exp, scale=-1.0)
            nc.scalar.activation(bC[:], Ef[:], ACTF.Relu,
                                 accum_out=slot(SL_BCE + 3))
            nc.scalar.activation(bA[:], bB[:], ACTF.Ln, bias=1.0,
                                 accum_out=slot(SL_BCE + 4))
            # sum E*m  (m in {0,1})
            nc.vector.scalar_tensor_tensor(
                bB[:], Mf[:], 1.0, Ef[:],
                op0=ALU.is_equal, op1=ALU.mult, accum_out=slot(SL_BCE + 0))
            # bce_pos_core = (relu + l1p) summed over m==1
            nc.vector.tensor_tensor(bB[:], bC[:], bA[:], op=ALU.add)
            nc.vector.scalar_tensor_tensor(
                bC[:], Mf[:], 1.0, bB[:],
                op0=ALU.is_equal, op1=ALU.mult, accum_out=slot(SL_BCE + 1))
            nc.vector.tensor_scalar(
                bA[:], Mf[:], 1.0, None, op0=ALU.is_equal, op1=ALU.add,
                accum_out=slot(SL_BCE + 2))

            nc.sync.dma_start(acc_d.ap()[:, :], accT[:])

    nc.finalize()
    return nc


_CACHE = {}


def _get_program():
    if "nc" not in _CACHE:
        _CACHE["nc"] = build_program()
    return _CACHE["nc"]


def make_in_maps(segin, edgein, segmask, edgemask):
    in_maps = []
    for k in range(NCORES):
        n, h = k // 2, k % 2
        rs = slice(h * (H // 2), (h + 1) * (H // 2))
        in_maps.append({
            "xs": np.ascontiguousarray(
                segin[n, :, rs, :].reshape(C, P, Q)),
            "ts": np.ascontiguousarray(
                segmask[n, rs, :].reshape(P, Q)),
            "es": np.ascontiguousarray(
                edgein[n, 0, rs, :].reshape(P, Q)),
            "ms": np.ascontiguousarray(
                edgemask[n, 0, rs, :].reshape(P, Q)),
        })
    return in_maps


def core_quants(part):
    """part: [SLOTS] f64 sums for one core -> (SA, SB, NA, NB, bce[5])."""
    SA = np.zeros(C)
    SB = np.zeros(C)
    NA = np.zeros(C)
    NB = np.zeros(C)
    for g in range(NPAIR):
        SA += part[SL_S + g * 38: SL_S + g * 38 + 19]
        SB += part[SL_S + g * 38 + 19: SL_S + g * 38 + 38]
    for h in range(2):
        NA += part[SL_N + h * 38: SL_N + h * 38 + 19]
        NB += part[SL_N + h * 38 + 19: SL_N + h * 38 + 38]
    return SA, SB, NA, NB, part[SL_BCE:SL_BCE + 5]


def combine(acc_list):
    """acc_list: per-core [P, SLOTS] arrays -> final f32 scalar loss."""
    part = np.zeros((NCORES, SLOTS))
    for k in range(NCORES):
        part[k] = acc_list[k].astype(np.float64).sum(axis=0)

    q = [core_quants(part[k]) for k in range(NCORES)]

    seg_loss = 0.0
    att_loss = 0.0
    for n in range(N):
        c0, c1 = 2 * n, 2 * n + 1
        S1 = q[c0][0] + q[c0][1] + q[c1][0] + q[c1][1]
        S2 = q[c0][1] + q[c1][1]
        bins = q[c0][2] + q[c0][3] + q[c1][2] + q[c1][3]
        bins2 = q[c0][3] + q[c1][3]

        w1 = (bins != 0) * (1.0 - bins / HW) + 1.0
        seg_loss += -(w1 * S1).sum() / (w1 * bins).sum()

        vsum = bins2.sum()
        w2 = (bins2 != 0) * (1.0 - bins2 / vsum) + 1.0
        att_loss += -(w2 * S2).sum() / (w2 * bins2).sum()

    bce = sum(q[k][4] for k in range(NCORES))
    sum_em, sum_b1m, pos_num, sum_relu, sum_l1p = bce

    # bce = relu(E) + ln1p(exp(-|E|)) - E*m
    all_bce = sum_relu + sum_l1p - sum_em
    pos_bce = sum_b1m - sum_em
    cnt = float(N * HW)
    neg_num = cnt - pos_num
    neg_bce = all_bce - pos_bce
    ssum = pos_num + neg_num
    edge_loss = (neg_num / ssum * pos_bce + pos_num / ssum * neg_bce) / cnt

    return np.float32(seg_loss + 0.3 * edge_loss + 0.1 * att_loss)


def run_cores(in_maps, trace=False, **kw):
    nc = _get_program()
    res = bass_utils.run_bass_kernel_spmd(
        nc, in_maps, core_ids=list(range(NCORES)), trace=trace, **kw
    )
    return res


def kernel(segin, edgein, segmask, edgemask):
    in_maps = make_in_maps(
        np.asarray(segin), np.asarray(edgein),
        np.asarray(segmask), np.asarray(edgemask))
    res = run_cores(in_maps)
    acc_list = [out["acc"] for out in res.results]
    return combine(acc_list)
